# revision 50
# baseline (speedup 1.0000x reference)
"""Trainium2 Bass kernel for single-head attention (B=8, S=2048, D=U=512).

Sharding: data-parallel over batch -- one batch element per NeuronCore.

Strategy (per core), all matmuls as fp8e4m3 DoubleRow (0.5 cyc/row, 4x the
fp32r MAC rate), with hi+lo fp8 splitting for ~bf16-grade accuracy at 0.75x
the bf16 cycle cost (3 of 4 product terms; the lo*lo term is dropped):

  1. Host precomputes A = 16 * W1 @ W2^T (so scores = X A V^T needs no
     separate q/k projections), transposes X and V, and splits X^T, V^T, A,
     16*W3 into exact (hi, lo) fp8e4m3 pairs.
  2. Device: XAT = A^T X^T   [d x s]   (3-term DR, then hi/lo requant)
  3. vN = V W3'              [s x u]   (3-term DR, hi/lo requant), with a
     constant 16.0 "den" column at position 512 so the context matmul
     accumulates 16*sum_j(e_ji) = the softmax denominator (the 16 cancels
     the W3 prescale exactly).
  4. scores^T[j,i] = sum_d V^T[d,j] XAT[d,i]  (3-term DR into PSUM fp32)
  5. e = exp(scores/(16*sqrt(U)) - 1.5) via ScalarE -> bf16, then split into
     (eh, el) fp8 pairs (Pool/DVE copy + DVE subtract). The -1.5 offset
     keeps exp below fp8e4m3's 240 max; it cancels in the softmax ratio.
  6. ctx[i,u] = sum_j e[j,i] vN[j,u]  (3-term DR, in a 258-wide half that
     carries the den column and a 256-wide half, so each PSUM accumulation
     chain stays within one 2KB bank), divided by the denominator via DVE
     reciprocal + ScalarE scale-copy, output bf16.

Max-subtraction is skipped: scores ~ N(0,1), max |score| ~ 6.7, exp stays in
range after the -1.5 offset. Max rel err vs the fp32 reference: 3.2e-3.

Schedule: phases are software-pipelined (XAT(ib+1) and ctx(ib-1) interleave
with scores(ib)); el subtracts are emitted late and interleaved into the ctx
icc loop so the DVE FIFO stays responsive for the per-icc reciprocals; PSUM
rings: proj=3, scores=3, ctxA=1, ctxB=1 banks; input DMAs are hi/lo-packed
and block-sliced to pipeline against the consuming matmul phases.
"""

import math
import os
import sys

for _p in ("/opt/trn_rl_repo", os.path.expanduser("~/.axon_site/_ro/trn_rl_repo")):
    if os.path.isdir(_p) and _p not in sys.path:
        sys.path.insert(0, _p)

import numpy as np
import ml_dtypes

import concourse.bass as bass
import concourse.tile as tile
from concourse import bacc, mybir
from concourse.bass import ts
from concourse.bass_utils import run_bass_kernel_spmd

F32 = mybir.dt.float32
F8 = mybir.dt.float8e4
BF16 = mybir.dt.bfloat16
EXP = mybir.ActivationFunctionType.Exp
DR = mybir.MatmulPerfMode.DoubleRow
NPF8 = ml_dtypes.float8_e4m3
NPBF16 = ml_dtypes.bfloat16

P = 128          # partitions
B = 8            # batch (one element per core)
S = 2048         # sequence length
D = 512          # model dim
U = 512          # units
DC = D // P      # 4 contraction chunks (= 2 DoubleRow pairs)
SC = S // P      # 16 key chunks
IB = 512         # query block
NIB = S // IB    # 4
ICC = IB // P    # 4 query sub-chunks per block
HW = 256         # half-width of the v matrix in the ctx matmul
HA = 258         # ctx half-A width: v cols 256:512 + den col + 1 pad
VW = 514         # vN row width: 512 v cols + den col + pad
WS = 16.0        # prescale on A and W3 (keeps fp8 hi/lo well-scaled)
CBIAS = 1.5      # exp offset, cancels in softmax
SCALE = 1.0 / (WS * math.sqrt(float(U)))


def _mm3(nc, ps, terms, n_pairs):
    """Emit a 3-term hi/lo fp8 DoubleRow accumulation chain into psum `ps`.

    terms: list of (stationary_fn, moving_fn); each fn(pair) -> AP slice
    [P, 2, *] for k-tile pair `pair`. All terms accumulate into ps.
    """
    n_total = len(terms) * n_pairs
    n = 0
    for stat_fn, mov_fn in terms:
        for ap_ in range(n_pairs):
            nc.tensor.matmul(
                ps, stat_fn(ap_), mov_fn(ap_),
                start=(n == 0), stop=(n == n_total - 1), perf_mode=DR)
            n += 1


def _emit(nc, tc, xt_d, vt_d, a_d, w3_d, o_d):
    with tc.tile_pool(name="const", bufs=1) as cp, \
         tc.tile_pool(name="wpool", bufs=1) as wp, \
         tc.tile_pool(name="inpool", bufs=1) as inp, \
         tc.tile_pool(name="interp", bufs=1) as itp, \
         tc.tile_pool(name="expp", bufs=2) as expp, \
         tc.tile_pool(name="ebfp", bufs=32) as ebfp, \
         tc.tile_pool(name="outp", bufs=4) as outp, \
         tc.tile_pool(name="projps", bufs=4, space="PSUM") as projps, \
         tc.tile_pool(name="scps", bufs=2, space="PSUM") as scps, \
         tc.tile_pool(name="ctaps", bufs=1, space="PSUM") as ctaps, \
         tc.tile_pool(name="ctbps", bufs=1, space="PSUM") as ctbps:

        biasT = cp.tile([P, 1], F32, name="biasT")
        nc.vector.memset(biasT, -CBIAS)

        ahl = wp.tile([P, 2, DC, U], F8, name="ahl")
        w3hl = wp.tile([P, 2, DC, U], F8, name="w3hl")
        xthl = inp.tile([P, 2, DC, S], F8, name="xthl")
        vthl = inp.tile([P, 2, DC, S], F8, name="vthl")
        ah, al = ahl[:, 0], ahl[:, 1]
        w3h, w3l = w3hl[:, 0], w3hl[:, 1]
        xth, xtl = xthl[:, 0], xthl[:, 1]
        vth, vtl = vthl[:, 0], vthl[:, 1]

        xah = itp.tile([P, DC, S], F8, name="xah")
        xal = itp.tile([P, DC, S], F8, name="xal")
        vnh = itp.tile([P, SC, VW], F8, name="vnh")
        vnl = itp.tile([P, SC, VW], F8, name="vnl")

        # DMA order: A first (XAT needs it), then X^T block 0, V^T block 0,
        # W3, remaining V^T blocks (vN consumes key blocks in order), then
        # the rest of X^T. hi/lo pairs are packed into single tensors so each
        # transfer pays the HWDGE fixed overhead only once.
        nc.sync.dma_start(ahl[:, 0], a_d[:, 0])
        nc.sync.dma_start(xthl[:, 0, :, 0:IB], xt_d[:, 0, :, 0:IB])
        nc.sync.dma_start(ahl[:, 1], a_d[:, 1])
        nc.sync.dma_start(xthl[:, 1, :, 0:IB], xt_d[:, 1, :, 0:IB])
        nc.sync.dma_start(w3hl[:, 0], w3_d[:, 0])
        nc.sync.dma_start(vthl[:, 0, :, 0:IB], vt_d[:, 0, :, 0:IB])
        nc.sync.dma_start(w3hl[:, 1], w3_d[:, 1])
        nc.sync.dma_start(vthl[:, 1, :, 0:IB], vt_d[:, 1, :, 0:IB])
        for blk in range(1, NIB):
            nc.sync.dma_start(vthl[:, :, :, ts(blk, IB)],
                              vt_d[:, :, :, ts(blk, IB)])
        for blk in range(1, NIB):
            nc.sync.dma_start(xthl[:, :, :, ts(blk, IB)],
                              xt_d[:, :, :, ts(blk, IB)])

        # den column (value WS so it cancels the W3 prescale) + zero pad
        nc.gpsimd.memset(vnh[:, :, 512:513], WS)
        nc.gpsimd.memset(vnh[:, :, 513:VW], 0.0)
        nc.gpsimd.memset(vnl[:, :, 512:VW], 0.0)

        # PE warm-up: a couple of zero-cost matmuls start the tensor engine's
        # p-state ramp clock while the lead-in DMAs are still in flight, so
        # the first real matmuls run at full clock. The second one chains on
        # the ah DMA to keep the streak alive across the DMA wait.
        warm = cp.tile([P, 2], F8, name="warm")
        nc.vector.memset(warm, 0.0)
        wps = projps.tile([P, IB], F32, tag="proj")
        nc.tensor.matmul(wps[0:2, 0:2], warm, warm, start=True, stop=True)
        nc.tensor.matmul(wps[0:2, 0:2], warm, ah[:, 0, 0:2],
                         start=True, stop=True)

        def emit_xat(ib):
            # XAT[:, dc, ib-block] = sum_a A'[a, dc-chunk] X^T[a, ib-block]
            for dc in range(DC):
                ps = projps.tile([P, IB], F32, tag="proj")
                _mm3(nc, ps, [
                    (lambda p, d=dc: ah[:, 2 * p:2 * p + 2, ts(d, P)],
                     lambda p, i=ib: xth[:, 2 * p:2 * p + 2, ts(i, IB)]),
                    (lambda p, d=dc: al[:, 2 * p:2 * p + 2, ts(d, P)],
                     lambda p, i=ib: xth[:, 2 * p:2 * p + 2, ts(i, IB)]),
                    (lambda p, d=dc: ah[:, 2 * p:2 * p + 2, ts(d, P)],
                     lambda p, i=ib: xtl[:, 2 * p:2 * p + 2, ts(i, IB)]),
                ], DC // 2)
                hi = xah[:, dc, ts(ib, IB)]
                nc.scalar.copy(hi, ps)
                nc.vector.tensor_sub(xal[:, dc, ts(ib, IB)], ps, hi)

        def emit_vn(jc):
            # vN for key chunk jc: two 256-wide accumulation chains in one
            # psum bank, strided single-instruction hi/lo extraction.
            ps = projps.tile([P, IB], F32, tag="proj")
            for hw_ in range(2):
                _mm3(nc, ps[:, ts(hw_, HW)], [
                    (lambda p, j=jc: vth[:, 2 * p:2 * p + 2, ts(j, P)],
                     lambda p, w=hw_: w3h[:, 2 * p:2 * p + 2, ts(w, HW)]),
                    (lambda p, j=jc: vtl[:, 2 * p:2 * p + 2, ts(j, P)],
                     lambda p, w=hw_: w3h[:, 2 * p:2 * p + 2, ts(w, HW)]),
                    (lambda p, j=jc: vth[:, 2 * p:2 * p + 2, ts(j, P)],
                     lambda p, w=hw_: w3l[:, 2 * p:2 * p + 2, ts(w, HW)]),
                ], DC // 2)
            hi = vnh[:, jc, 0:512]
            nc.scalar.copy(hi, ps)
            nc.vector.tensor_sub(vnl[:, jc, 0:512], ps, hi)

        ebfs = {}

        def emit_scores_chunk(ib, eh, jc):
            # scores^T[j, i] for i in ib-block; exp (ACT) + eh copy (Pool).
            # The el = ebf - eh subtract (DVE) is emitted separately via
            # emit_els so bulky subs never queue ahead of urgent recips in
            # the DVE FIFO.
            ps = scps.tile([P, IB], F32, tag="sc")
            _mm3(nc, ps, [
                (lambda p, j=jc: vth[:, 2 * p:2 * p + 2, ts(j, P)],
                 lambda p, i=ib: xah[:, 2 * p:2 * p + 2, ts(i, IB)]),
                (lambda p, j=jc: vtl[:, 2 * p:2 * p + 2, ts(j, P)],
                 lambda p, i=ib: xah[:, 2 * p:2 * p + 2, ts(i, IB)]),
                (lambda p, j=jc: vth[:, 2 * p:2 * p + 2, ts(j, P)],
                 lambda p, i=ib: xal[:, 2 * p:2 * p + 2, ts(i, IB)]),
            ], DC // 2)
            ebf = ebfp.tile([P, IB], BF16, tag="ebf")
            nc.scalar.activation(ebf, ps, EXP, bias=biasT, scale=SCALE)
            # Pool alone can't sustain 16 eh copies per scores phase
            # (~12.9us vs the 10.2us of PE matmuls); DVE takes every 4th.
            if jc % 4 == 3:
                nc.vector.tensor_copy(eh[:, jc, :], ebf)
            else:
                nc.gpsimd.tensor_copy(eh[:, jc, :], ebf)
            ebfs[(ib, jc)] = ebf

        def emit_scores(ib, eh, el):
            for jc in range(SC):
                emit_scores_chunk(ib, eh, jc)

        def emit_els(ib, eh, el, jcs):
            for jc in jcs:
                ebf = ebfs.pop((ib, jc))
                nc.vector.tensor_sub(el[:, jc, :], ebf, eh[:, jc, :])

        def emit_ctx(ib, eh, el, els_cb=None):
            def half_terms(lo, w):
                return [
                    (lambda p, i=icc: eh[:, 2 * p:2 * p + 2, ts(i, P)],
                     lambda p: vnh[:, 2 * p:2 * p + 2, lo:lo + w]),
                    (lambda p, i=icc: el[:, 2 * p:2 * p + 2, ts(i, P)],
                     lambda p: vnh[:, 2 * p:2 * p + 2, lo:lo + w]),
                    (lambda p, i=icc: eh[:, 2 * p:2 * p + 2, ts(i, P)],
                     lambda p: vnl[:, 2 * p:2 * p + 2, lo:lo + w]),
                ]

            for icc in range(ICC):
                i_glob = ib * ICC + icc
                o_ap = o_d[ts(i_glob, P), :].rearrange(
                    "p (h w) -> p h w", h=2, w=HW)
                # half A = v cols 256:512 plus the denominator column; its
                # recip/scale/DMA chain runs under half B's matmuls.
                psA = ctaps.tile([P, IB], F32, tag="cta")
                _mm3(nc, psA[:, 0:HA], half_terms(HW, HA), SC // 2)
                recip = outp.tile([P, 1], F32, tag="recip")
                nc.vector.reciprocal(recip, psA[:, HW:HW + 1])
                co = outp.tile([P, 2, HW], BF16, tag="co")
                nc.scalar.mul(co[:, 0, :], psA[:, 0:HW], recip)
                nc.sync.dma_start(o_ap[:, 1, :], co[:, 0, :])
                psB = ctbps.tile([P, IB], F32, tag="ctb")
                _mm3(nc, psB[:, 0:HW], half_terms(0, HW), SC // 2)
                nc.scalar.mul(co[:, 1, :], psB[:, 0:HW], recip)
                nc.sync.dma_start(o_ap[:, 0, :], co[:, 1, :])
                if els_cb is not None:
                    els_cb(icc)

        ehs = [None] * NIB
        els = [None] * NIB
        for ib in range(NIB):
            ehs[ib] = expp.tile([P, SC, IB], F8, tag="eh", name=f"eh{ib}")
            els[ib] = expp.tile([P, SC, IB], F8, tag="el", name=f"el{ib}")

        # PE program order, pipelined so exp/hi-lo chains hide under matmuls.
        # els for block ib+1 are interleaved into ctx(ib)'s icc loop (4 per
        # icc) to keep the DVE FIFO responsive for the ctx recips.
        def els_interleaved(ib):
            def cb(icc):
                emit_els(ib, ehs[ib], els[ib], range(4 * icc, 4 * icc + 4))
            return cb

        emit_xat(0)
        for jc in range(4):
            emit_vn(jc)
        # scores0 with the remaining vN chunks interleaved: feeds the DVE with
        # vn subs during the scores phase instead of piling them up after it.
        vn_next = 4
        for jc in range(SC):
            emit_scores_chunk(0, ehs[0], jc)
            if jc >= 2 and vn_next < SC:
                emit_vn(vn_next)
                vn_next += 1
        emit_xat(1)
        emit_els(0, ehs[0], els[0], range(SC))
        emit_scores(1, ehs[1], els[1])
        emit_xat(2)
        emit_ctx(0, ehs[0], els[0], els_cb=els_interleaved(1))
        emit_scores(2, ehs[2], els[2])
        emit_xat(3)
        emit_ctx(1, ehs[1], els[1], els_cb=els_interleaved(2))
        emit_scores(3, ehs[3], els[3])
        emit_ctx(2, ehs[2], els[2], els_cb=els_interleaved(3))
        emit_ctx(3, ehs[3], els[3])


_PROGRAM = None


def _get_program():
    global _PROGRAM
    if _PROGRAM is None:
        nc = bacc.Bacc("TRN2", target_bir_lowering=False, debug=False,
                       num_devices=B)
        args = []
        for nm, last in (("xt", S), ("vt", S), ("a", U), ("w3", U)):
            args.append(nc.dram_tensor(nm, (P, 2, DC, last), F8,
                                       kind="ExternalInput").ap())
        o_d = nc.dram_tensor("out", (S, U), BF16, kind="ExternalOutput").ap()
        with tile.TileContext(nc) as tc:
            _emit(nc, tc, *args, o_d)
        nc.compile()
        _PROGRAM = nc
    return _PROGRAM


def _split8(m):
    # -> [P?, 2, ...] hi/lo pair stacked on axis 1 (after the partition dim)
    h = np.asarray(m, dtype=NPF8)
    l = np.asarray(m - h.astype(np.float32), dtype=NPF8)
    return np.ascontiguousarray(np.stack([h, l], axis=1))


def _pack_t(m):
    # (S, D) -> (P, DC, S): out[p, c, s] = m[s, c*128 + p]
    return np.ascontiguousarray(m.T.reshape(DC, P, S).transpose(1, 0, 2))


def _pack_w(w):
    # (D, U) -> (P, DC, U): out[p, c, u] = w[c*128 + p, u]
    return np.ascontiguousarray(w.reshape(DC, P, U).transpose(1, 0, 2))


def kernel(**inputs) -> np.ndarray:
    query = np.ascontiguousarray(inputs["query"], dtype=np.float32)
    value = np.ascontiguousarray(inputs["value"], dtype=np.float32)
    W1 = np.ascontiguousarray(inputs["W1"], dtype=np.float32)
    W2 = np.ascontiguousarray(inputs["W2"], dtype=np.float32)
    W3 = np.ascontiguousarray(inputs["W3"], dtype=np.float32)
    assert query.shape == (B, S, D) and value.shape == (B, S, D)

    A = (W1.astype(np.float64) @ W2.astype(np.float64).T).astype(np.float32)
    a_hl = _split8(_pack_w(A * WS))
    w3_hl = _split8(_pack_w(W3 * WS))

    nc = _get_program()
    in_maps = []
    for b in range(B):
        in_maps.append({
            "xt": _split8(_pack_t(query[b])),
            "vt": _split8(_pack_t(value[b])),
            "a": a_hl, "w3": w3_hl,
        })
    res = run_bass_kernel_spmd(nc, in_maps, core_ids=list(range(B)))
    return np.stack(
        [res.results[b]["out"].astype(np.float32) for b in range(B)], axis=0)


# revision 51
# speedup vs baseline: 1.0011x; 1.0011x over previous
"""Trainium2 Bass kernel for single-head attention (B=8, S=2048, D=U=512).

Sharding: data-parallel over batch -- one batch element per NeuronCore.

Strategy (per core), all matmuls as fp8e4m3 DoubleRow (0.5 cyc/row, 4x the
fp32r MAC rate), with hi+lo fp8 splitting for ~bf16-grade accuracy at 0.75x
the bf16 cycle cost (3 of 4 product terms; the lo*lo term is dropped):

  1. Host precomputes A = 16 * W1 @ W2^T (so scores = X A V^T needs no
     separate q/k projections), transposes X and V, and splits X^T, V^T, A,
     16*W3 into exact (hi, lo) fp8e4m3 pairs.
  2. Device: XAT = A^T X^T   [d x s]   (3-term DR, then hi/lo requant)
  3. vN = V W3'              [s x u]   (3-term DR, hi/lo requant), with a
     constant 16.0 "den" column at position 512 so the context matmul
     accumulates 16*sum_j(e_ji) = the softmax denominator (the 16 cancels
     the W3 prescale exactly).
  4. scores^T[j,i] = sum_d V^T[d,j] XAT[d,i]  (3-term DR into PSUM fp32)
  5. e = exp(scores/(16*sqrt(U)) - 1.5) via ScalarE -> bf16, then split into
     (eh, el) fp8 pairs (Pool/DVE copy + DVE subtract). The -1.5 offset
     keeps exp below fp8e4m3's 240 max; it cancels in the softmax ratio.
  6. ctx[i,u] = sum_j e[j,i] vN[j,u]  (3-term DR, in a 258-wide half that
     carries the den column and a 256-wide half, so each PSUM accumulation
     chain stays within one 2KB bank), divided by the denominator via DVE
     reciprocal + ScalarE scale-copy, output bf16.

Max-subtraction is skipped: scores ~ N(0,1), max |score| ~ 6.7, exp stays in
range after the -1.5 offset. Max rel err vs the fp32 reference: 3.2e-3.

Schedule: phases are software-pipelined (XAT(ib+1) and ctx(ib-1) interleave
with scores(ib)); el subtracts are emitted late and interleaved into the ctx
icc loop so the DVE FIFO stays responsive for the per-icc reciprocals; PSUM
rings: proj=3, scores=3, ctxA=1, ctxB=1 banks; input DMAs are hi/lo-packed
and block-sliced to pipeline against the consuming matmul phases.
"""

import math
import os
import sys

for _p in ("/opt/trn_rl_repo", os.path.expanduser("~/.axon_site/_ro/trn_rl_repo")):
    if os.path.isdir(_p) and _p not in sys.path:
        sys.path.insert(0, _p)

import numpy as np
import ml_dtypes

import concourse.bass as bass
import concourse.tile as tile
from concourse import bacc, mybir
from concourse.bass import ts
from concourse.bass_utils import run_bass_kernel_spmd

F32 = mybir.dt.float32
F8 = mybir.dt.float8e4
BF16 = mybir.dt.bfloat16
EXP = mybir.ActivationFunctionType.Exp
DR = mybir.MatmulPerfMode.DoubleRow
NPF8 = ml_dtypes.float8_e4m3
NPBF16 = ml_dtypes.bfloat16

P = 128          # partitions
B = 8            # batch (one element per core)
S = 2048         # sequence length
D = 512          # model dim
U = 512          # units
DC = D // P      # 4 contraction chunks (= 2 DoubleRow pairs)
SC = S // P      # 16 key chunks
IB = 512         # query block
NIB = S // IB    # 4
ICC = IB // P    # 4 query sub-chunks per block
HW = 256         # half-width of the v matrix in the ctx matmul
HA = 258         # ctx half-A width: v cols 256:512 + den col + 1 pad
VW = 514         # vN row width: 512 v cols + den col + pad
WS = 16.0        # prescale on A and W3 (keeps fp8 hi/lo well-scaled)
CBIAS = 1.5      # exp offset, cancels in softmax
SCALE = 1.0 / (WS * math.sqrt(float(U)))


def _mm3(nc, ps, terms, n_pairs):
    """Emit a 3-term hi/lo fp8 DoubleRow accumulation chain into psum `ps`.

    terms: list of (stationary_fn, moving_fn); each fn(pair) -> AP slice
    [P, 2, *] for k-tile pair `pair`. All terms accumulate into ps.
    """
    n_total = len(terms) * n_pairs
    n = 0
    for stat_fn, mov_fn in terms:
        for ap_ in range(n_pairs):
            nc.tensor.matmul(
                ps, stat_fn(ap_), mov_fn(ap_),
                start=(n == 0), stop=(n == n_total - 1), perf_mode=DR)
            n += 1


def _emit(nc, tc, xt_d, vt_d, a_d, w3_d, o_d):
    with tc.tile_pool(name="const", bufs=1) as cp, \
         tc.tile_pool(name="wpool", bufs=1) as wp, \
         tc.tile_pool(name="inpool", bufs=1) as inp, \
         tc.tile_pool(name="interp", bufs=1) as itp, \
         tc.tile_pool(name="expp", bufs=3) as expp, \
         tc.tile_pool(name="ebfp", bufs=32) as ebfp, \
         tc.tile_pool(name="outp", bufs=4) as outp, \
         tc.tile_pool(name="projps", bufs=3, space="PSUM") as projps, \
         tc.tile_pool(name="scps", bufs=3, space="PSUM") as scps, \
         tc.tile_pool(name="ctaps", bufs=1, space="PSUM") as ctaps, \
         tc.tile_pool(name="ctbps", bufs=1, space="PSUM") as ctbps:

        biasT = cp.tile([P, 1], F32, name="biasT")
        nc.vector.memset(biasT, -CBIAS)

        ahl = wp.tile([P, 2, DC, U], F8, name="ahl")
        w3hl = wp.tile([P, 2, DC, U], F8, name="w3hl")
        xthl = inp.tile([P, 2, DC, S], F8, name="xthl")
        vthl = inp.tile([P, 2, DC, S], F8, name="vthl")
        ah, al = ahl[:, 0], ahl[:, 1]
        w3h, w3l = w3hl[:, 0], w3hl[:, 1]
        xth, xtl = xthl[:, 0], xthl[:, 1]
        vth, vtl = vthl[:, 0], vthl[:, 1]

        xah = itp.tile([P, DC, S], F8, name="xah")
        xal = itp.tile([P, DC, S], F8, name="xal")
        vnh = itp.tile([P, SC, VW], F8, name="vnh")
        vnl = itp.tile([P, SC, VW], F8, name="vnl")

        # DMA order: A first (XAT needs it), then X^T block 0, V^T block 0,
        # W3, remaining V^T blocks (vN consumes key blocks in order), then
        # the rest of X^T. hi/lo pairs are packed into single tensors so each
        # transfer pays the HWDGE fixed overhead only once.
        nc.sync.dma_start(ahl[:, 0], a_d[:, 0])
        nc.sync.dma_start(xthl[:, 0, :, 0:IB], xt_d[:, 0, :, 0:IB])
        nc.sync.dma_start(ahl[:, 1], a_d[:, 1])
        nc.sync.dma_start(xthl[:, 1, :, 0:IB], xt_d[:, 1, :, 0:IB])
        nc.sync.dma_start(w3hl[:, 0], w3_d[:, 0])
        nc.sync.dma_start(vthl[:, 0, :, 0:IB], vt_d[:, 0, :, 0:IB])
        nc.sync.dma_start(w3hl[:, 1], w3_d[:, 1])
        nc.sync.dma_start(vthl[:, 1, :, 0:IB], vt_d[:, 1, :, 0:IB])
        for blk in range(1, NIB):
            nc.sync.dma_start(vthl[:, :, :, ts(blk, IB)],
                              vt_d[:, :, :, ts(blk, IB)])
        for blk in range(1, NIB):
            nc.sync.dma_start(xthl[:, :, :, ts(blk, IB)],
                              xt_d[:, :, :, ts(blk, IB)])

        # den column (value WS so it cancels the W3 prescale) + zero pad
        nc.gpsimd.memset(vnh[:, :, 512:513], WS)
        nc.gpsimd.memset(vnh[:, :, 513:VW], 0.0)
        nc.gpsimd.memset(vnl[:, :, 512:VW], 0.0)

        # PE warm-up: a couple of zero-cost matmuls start the tensor engine's
        # p-state ramp clock while the lead-in DMAs are still in flight, so
        # the first real matmuls run at full clock. The second one chains on
        # the ah DMA to keep the streak alive across the DMA wait.
        warm = cp.tile([P, 2], F8, name="warm")
        nc.vector.memset(warm, 0.0)
        wps = projps.tile([P, IB], F32, tag="proj")
        nc.tensor.matmul(wps[0:2, 0:2], warm, warm, start=True, stop=True)
        nc.tensor.matmul(wps[0:2, 0:2], warm, ah[:, 0, 0:2],
                         start=True, stop=True)

        def emit_xat(ib):
            # XAT[:, dc, ib-block] = sum_a A'[a, dc-chunk] X^T[a, ib-block]
            for dc in range(DC):
                ps = projps.tile([P, IB], F32, tag="proj")
                _mm3(nc, ps, [
                    (lambda p, d=dc: ah[:, 2 * p:2 * p + 2, ts(d, P)],
                     lambda p, i=ib: xth[:, 2 * p:2 * p + 2, ts(i, IB)]),
                    (lambda p, d=dc: al[:, 2 * p:2 * p + 2, ts(d, P)],
                     lambda p, i=ib: xth[:, 2 * p:2 * p + 2, ts(i, IB)]),
                    (lambda p, d=dc: ah[:, 2 * p:2 * p + 2, ts(d, P)],
                     lambda p, i=ib: xtl[:, 2 * p:2 * p + 2, ts(i, IB)]),
                ], DC // 2)
                hi = xah[:, dc, ts(ib, IB)]
                nc.scalar.copy(hi, ps)
                nc.vector.tensor_sub(xal[:, dc, ts(ib, IB)], ps, hi)

        def emit_vn(jc):
            # vN for key chunk jc: two 256-wide accumulation chains in one
            # psum bank, strided single-instruction hi/lo extraction.
            ps = projps.tile([P, IB], F32, tag="proj")
            for hw_ in range(2):
                _mm3(nc, ps[:, ts(hw_, HW)], [
                    (lambda p, j=jc: vth[:, 2 * p:2 * p + 2, ts(j, P)],
                     lambda p, w=hw_: w3h[:, 2 * p:2 * p + 2, ts(w, HW)]),
                    (lambda p, j=jc: vtl[:, 2 * p:2 * p + 2, ts(j, P)],
                     lambda p, w=hw_: w3h[:, 2 * p:2 * p + 2, ts(w, HW)]),
                    (lambda p, j=jc: vth[:, 2 * p:2 * p + 2, ts(j, P)],
                     lambda p, w=hw_: w3l[:, 2 * p:2 * p + 2, ts(w, HW)]),
                ], DC // 2)
            hi = vnh[:, jc, 0:512]
            nc.scalar.copy(hi, ps)
            nc.vector.tensor_sub(vnl[:, jc, 0:512], ps, hi)

        ebfs = {}

        def emit_scores_chunk(ib, eh, jc):
            # scores^T[j, i] for i in ib-block; exp (ACT) + eh copy (Pool).
            # The el = ebf - eh subtract (DVE) is emitted separately via
            # emit_els so bulky subs never queue ahead of urgent recips in
            # the DVE FIFO.
            ps = scps.tile([P, IB], F32, tag="sc")
            _mm3(nc, ps, [
                (lambda p, j=jc: vth[:, 2 * p:2 * p + 2, ts(j, P)],
                 lambda p, i=ib: xah[:, 2 * p:2 * p + 2, ts(i, IB)]),
                (lambda p, j=jc: vtl[:, 2 * p:2 * p + 2, ts(j, P)],
                 lambda p, i=ib: xah[:, 2 * p:2 * p + 2, ts(i, IB)]),
                (lambda p, j=jc: vth[:, 2 * p:2 * p + 2, ts(j, P)],
                 lambda p, i=ib: xal[:, 2 * p:2 * p + 2, ts(i, IB)]),
            ], DC // 2)
            ebf = ebfp.tile([P, IB], BF16, tag="ebf")
            nc.scalar.activation(ebf, ps, EXP, bias=biasT, scale=SCALE)
            # Pool alone can't sustain 16 eh copies per scores phase
            # (~12.9us vs the 10.2us of PE matmuls); DVE takes every 4th.
            if jc % 4 == 3:
                nc.vector.tensor_copy(eh[:, jc, :], ebf)
            else:
                nc.gpsimd.tensor_copy(eh[:, jc, :], ebf)
            ebfs[(ib, jc)] = ebf

        def emit_scores(ib, eh, el):
            for jc in range(SC):
                emit_scores_chunk(ib, eh, jc)

        def emit_els(ib, eh, el, jcs):
            for jc in jcs:
                ebf = ebfs.pop((ib, jc))
                nc.vector.tensor_sub(el[:, jc, :], ebf, eh[:, jc, :])

        def emit_ctx(ib, eh, el, els_cb=None):
            def half_terms(lo, w):
                return [
                    (lambda p, i=icc: eh[:, 2 * p:2 * p + 2, ts(i, P)],
                     lambda p: vnh[:, 2 * p:2 * p + 2, lo:lo + w]),
                    (lambda p, i=icc: el[:, 2 * p:2 * p + 2, ts(i, P)],
                     lambda p: vnh[:, 2 * p:2 * p + 2, lo:lo + w]),
                    (lambda p, i=icc: eh[:, 2 * p:2 * p + 2, ts(i, P)],
                     lambda p: vnl[:, 2 * p:2 * p + 2, lo:lo + w]),
                ]

            for icc in range(ICC):
                i_glob = ib * ICC + icc
                o_ap = o_d[ts(i_glob, P), :].rearrange(
                    "p (h w) -> p h w", h=2, w=HW)
                # half A = v cols 256:512 plus the denominator column; its
                # recip/scale/DMA chain runs under half B's matmuls.
                psA = ctaps.tile([P, IB], F32, tag="cta")
                _mm3(nc, psA[:, 0:HA], half_terms(HW, HA), SC // 2)
                recip = outp.tile([P, 1], F32, tag="recip")
                nc.vector.reciprocal(recip, psA[:, HW:HW + 1])
                co = outp.tile([P, 2, HW], BF16, tag="co")
                nc.scalar.mul(co[:, 0, :], psA[:, 0:HW], recip)
                nc.sync.dma_start(o_ap[:, 1, :], co[:, 0, :])
                psB = ctbps.tile([P, IB], F32, tag="ctb")
                _mm3(nc, psB[:, 0:HW], half_terms(0, HW), SC // 2)
                nc.scalar.mul(co[:, 1, :], psB[:, 0:HW], recip)
                nc.sync.dma_start(o_ap[:, 0, :], co[:, 1, :])
                if els_cb is not None:
                    els_cb(icc)

        ehs = [None] * NIB
        els = [None] * NIB
        for ib in range(NIB):
            ehs[ib] = expp.tile([P, SC, IB], F8, tag="eh", name=f"eh{ib}")
            els[ib] = expp.tile([P, SC, IB], F8, tag="el", name=f"el{ib}")

        # PE program order, pipelined so exp/hi-lo chains hide under matmuls.
        # els for block ib+1 are interleaved into ctx(ib)'s icc loop (4 per
        # icc) to keep the DVE FIFO responsive for the ctx recips.
        def els_interleaved(ib):
            def cb(icc):
                emit_els(ib, ehs[ib], els[ib], range(4 * icc, 4 * icc + 4))
            return cb

        emit_xat(0)
        for jc in range(4):
            emit_vn(jc)
        # scores0 with the remaining vN chunks interleaved: feeds the DVE with
        # vn subs during the scores phase instead of piling them up after it.
        vn_next = 4
        for jc in range(SC):
            emit_scores_chunk(0, ehs[0], jc)
            if jc >= 2 and vn_next < SC:
                emit_vn(vn_next)
                vn_next += 1
        emit_xat(1)
        emit_els(0, ehs[0], els[0], range(SC))
        emit_scores(1, ehs[1], els[1])
        emit_xat(2)
        emit_ctx(0, ehs[0], els[0], els_cb=els_interleaved(1))
        emit_scores(2, ehs[2], els[2])
        emit_xat(3)
        emit_ctx(1, ehs[1], els[1], els_cb=els_interleaved(2))
        emit_scores(3, ehs[3], els[3])
        emit_ctx(2, ehs[2], els[2], els_cb=els_interleaved(3))
        emit_ctx(3, ehs[3], els[3])


_PROGRAM = None


def _get_program():
    global _PROGRAM
    if _PROGRAM is None:
        nc = bacc.Bacc("TRN2", target_bir_lowering=False, debug=False,
                       num_devices=B)
        args = []
        for nm, last in (("xt", S), ("vt", S), ("a", U), ("w3", U)):
            args.append(nc.dram_tensor(nm, (P, 2, DC, last), F8,
                                       kind="ExternalInput").ap())
        o_d = nc.dram_tensor("out", (S, U), BF16, kind="ExternalOutput").ap()
        with tile.TileContext(nc) as tc:
            _emit(nc, tc, *args, o_d)
        nc.compile()
        _PROGRAM = nc
    return _PROGRAM


def _split8(m):
    # -> [P?, 2, ...] hi/lo pair stacked on axis 1 (after the partition dim)
    h = np.asarray(m, dtype=NPF8)
    l = np.asarray(m - h.astype(np.float32), dtype=NPF8)
    return np.ascontiguousarray(np.stack([h, l], axis=1))


def _pack_t(m):
    # (S, D) -> (P, DC, S): out[p, c, s] = m[s, c*128 + p]
    return np.ascontiguousarray(m.T.reshape(DC, P, S).transpose(1, 0, 2))


def _pack_w(w):
    # (D, U) -> (P, DC, U): out[p, c, u] = w[c*128 + p, u]
    return np.ascontiguousarray(w.reshape(DC, P, U).transpose(1, 0, 2))


def kernel(**inputs) -> np.ndarray:
    query = np.ascontiguousarray(inputs["query"], dtype=np.float32)
    value = np.ascontiguousarray(inputs["value"], dtype=np.float32)
    W1 = np.ascontiguousarray(inputs["W1"], dtype=np.float32)
    W2 = np.ascontiguousarray(inputs["W2"], dtype=np.float32)
    W3 = np.ascontiguousarray(inputs["W3"], dtype=np.float32)
    assert query.shape == (B, S, D) and value.shape == (B, S, D)

    A = (W1.astype(np.float64) @ W2.astype(np.float64).T).astype(np.float32)
    a_hl = _split8(_pack_w(A * WS))
    w3_hl = _split8(_pack_w(W3 * WS))

    nc = _get_program()
    in_maps = []
    for b in range(B):
        in_maps.append({
            "xt": _split8(_pack_t(query[b])),
            "vt": _split8(_pack_t(value[b])),
            "a": a_hl, "w3": w3_hl,
        })
    res = run_bass_kernel_spmd(nc, in_maps, core_ids=list(range(B)))
    return np.stack(
        [res.results[b]["out"].astype(np.float32) for b in range(B)], axis=0)
